# revision 53
# baseline (speedup 1.0000x reference)
"""Trainium2 Bass kernel for nn_Attention_48498770706573.

Fused QKV-projection + masked softmax attention, sharded over 8 NeuronCores:
data-parallel over batch (B=2), tensor-parallel over heads (16 -> 4 per
core). Host does slicing/transposition/constant-upload only.

v2 design (vs baseline):
  - projections consume f32r DRAM tiles directly (no bf16 pre-casts on DVE)
  - scores in fp8e4m3 DoubleRow (K=64 packed as 32x2) -> 0.5 cycles/col
  - rowsum fused into PV as a 65th "ones" column of vt (no ones-matmuls)
  - V-bias folded into vt via a K=1 rank-1 matmul during the V projection
    (out = (pv + bv*rowsum)/rowsum == pv'/rowsum with v' = v + bv)
  - tail: reciprocal_approx_fast on the psum rowsum row, rank-1 broadcast
    matmul, one DVE multiply per head
  - mask stays a DVE broadcast multiply on bf16 p tiles (TensorTensor 2x)
"""

import os

import numpy as np

import concourse.bacc as bacc
import concourse.bass_utils as _bu
import concourse.mybir as mybir
import concourse.tile as tile
from concourse.bass_utils import run_bass_kernel_spmd

_ = _bu  # walrus --enable-ldw-opt=true rejects bass-emitted InstLdweights

B, NQ, NK, D, H = 2, 2048, 2048, 1024, 16
DH = D // H  # 64
N_CORES = 8
HPC = H // (N_CORES // B)  # heads per core = 4
JW = HPC * DH  # per-core projection width = 256
NKT = NK // 128  # 16 nk tiles
NCH = 4  # nq chunks
CHW = NQ // NCH  # 512
DT = 8  # contraction d-tiles
LAG = 3

SCORES_FP8 = True

f32 = mybir.dt.float32
f32r = mybir.dt.float32r
bf16 = mybir.dt.bfloat16
fp8 = mybir.dt.float8e4
u8 = mybir.dt.uint8

QK_DT = fp8 if SCORES_FP8 else bf16


def _build():
    nc = bacc.Bacc(
        "TRN2", target_bir_lowering=False, debug=False, num_devices=N_CORES
    )

    qT = nc.dram_tensor("qT", [D, NQ], f32r, kind="ExternalInput")
    kT = nc.dram_tensor("kT", [D, NK], f32r, kind="ExternalInput")
    vT = nc.dram_tensor("vT", [D, NK], f32r, kind="ExternalInput")
    maskT = nc.dram_tensor("maskT", [NK, NQ], u8, kind="ExternalInput")
    wqT = nc.dram_tensor("wqT", [D, JW], f32r, kind="ExternalInput")
    wkT = nc.dram_tensor("wkT", [D, JW], f32r, kind="ExternalInput")
    wvT = nc.dram_tensor("wvT", [D, JW], f32r, kind="ExternalInput")
    bqd = nc.dram_tensor("bq", [2, 128], f32, kind="ExternalInput")
    bkd = nc.dram_tensor("bk", [2, 128], f32, kind="ExternalInput")
    bvrd = nc.dram_tensor("bvrow", [1, JW], f32, kind="ExternalInput")
    onesd = nc.dram_tensor("onesr", [1, 128], bf16, kind="ExternalInput")
    vtonesd = nc.dram_tensor("vtones", [128, NKT * HPC], bf16, kind="ExternalInput")
    o = nc.dram_tensor("o", [JW, NQ], f32, kind="ExternalOutput")
    KDBG = bool(int(os.environ.get("KDBG", "0")))
    if KDBG:
        dbg_vt = nc.dram_tensor(
            "dbg_vt", [128, NKT * HPC * 65], f32, kind="ExternalOutput"
        )
        dbg_pv = nc.dram_tensor(
            "dbg_pv", [128, HPC * CHW], f32, kind="ExternalOutput"
        )
        dbg_p = nc.dram_tensor(
            "dbg_p", [128, HPC * CHW], f32, kind="ExternalOutput"
        )
        dbg_rc = nc.dram_tensor(
            "dbg_rc", [2, HPC * CHW], f32, kind="ExternalOutput"
        )
        dbg_rb = nc.dram_tensor(
            "dbg_rb", [64, 2 * CHW], f32, kind="ExternalOutput"
        )

    with tile.TileContext(nc) as tc:
        with (
            tc.tile_pool(name="consts", bufs=1) as consts,
            tc.tile_pool(name="stage", bufs=12) as stage,
            tc.tile_pool(name="qpool", bufs=11) as qpool,
            tc.tile_pool(name="m8pool", bufs=16) as m8pool,
            tc.tile_pool(name="mbpool", bufs=6) as mbpool,
            tc.tile_pool(name="projout", bufs=1) as projout,
            tc.tile_pool(name="ppool", bufs=4) as ppool,
            tc.tile_pool(name="rcpool", bufs=1) as rcpool,
            tc.tile_pool(name="rbpool", bufs=2) as rbpool,
            tc.tile_pool(name="outsb", bufs=4) as outsb,
            tc.tile_pool(name="dbgpool", bufs=2) as dbgpool,
            tc.tile_pool(name="vbfp", bufs=10) as vbfp,
            tc.tile_pool(name="kbfp", bufs=10) as kbfp,
            tc.tile_pool(name="wtmp", bufs=1) as wtmp,
            tc.tile_pool(name="sps", bufs=2, space="PSUM") as sps,
            tc.tile_pool(name="pvps", bufs=1, space="PSUM") as pvps,
        ):
            # ---- constants ----
            wk_sb = wtmp.tile([128, DT, JW], f32r, tag="wt", name="wk_sb")
            for d in range(DT):
                nc.sync.dma_start(wk_sb[:, d], wkT[d * 128 : (d + 1) * 128, :])
            bq_sb = consts.tile([128, 2], f32, tag="bq")
            bk_sb = consts.tile([128, 2], f32, tag="bk")
            for m in range(2):
                nc.sync.dma_start(
                    bq_sb[:, m : m + 1],
                    bqd[m : m + 1, :].rearrange("a b -> b a"),
                )
                nc.sync.dma_start(
                    bk_sb[:, m : m + 1],
                    bkd[m : m + 1, :].rearrange("a b -> b a"),
                )
            bvrf = wtmp.tile([1, JW], f32, tag="bvrf", name="bvrf")
            nc.sync.dma_start(bvrf, bvrd[:])
            bvr_sb = consts.tile([1, JW], bf16, tag="bvr")
            nc.vector.tensor_copy(bvr_sb, bvrf)
            ones_sb = consts.tile([1, 128], bf16, tag="ones")
            nc.sync.dma_start(ones_sb, onesd[:])
            # bf16 copies of wq/wk for PE stationaries (ldw-opt compatible)
            wkb = consts.tile([128, DT, JW], bf16, tag="wkb")
            nc.vector.tensor_copy(wkb, wk_sb)

            # vt: [128, NKT, 4*65]; col 65h+64 is the ones column (rowsum)
            vt = projout.tile([128, NKT, HPC * 65], bf16, tag="vt")
            vt_ones_view = vt.rearrange("p n (h c) -> p n h c", c=65)[
                :, :, :, 64:65
            ].rearrange("p n h c -> p (n h c)")
            nc.sync.dma_start(vt_ones_view, vtonesd[:])

            # ---- decoupled input DMAs (priority order) ----
            def dma_x_chunk(src, ch, tiles=None, pool=None):
                pool = pool or stage
                tiles = {} if tiles is None else tiles
                for d in range(DT):
                    x = pool.tile([128, CHW], f32r, tag="xc", name="x")
                    nc.sync.dma_start(
                        x,
                        src[d * 128 : (d + 1) * 128, ch * CHW : (ch + 1) * CHW],
                    )
                    tiles[(d, ch)] = x
                return tiles

            k_tiles = {}
            for ch in range(NCH):
                x = stage.tile([128, CHW], f32r, tag="xc", name="x")
                nc.sync.dma_start(x, kT[0:128, ch * CHW : (ch + 1) * CHW])
                k_tiles[(0, ch)] = x
            wq_sb = wtmp.tile([128, DT, JW], f32r, tag="wt", name="wq_sb")
            for d in range(DT):
                nc.sync.dma_start(wq_sb[:, d], wqT[d * 128 : (d + 1) * 128, :])
            wqb = consts.tile([128, DT, JW], bf16, tag="wqb")
            nc.vector.tensor_copy(wqb, wq_sb)
            wv_sb = wtmp.tile([128, DT, JW], f32r, tag="wt", name="wv_sb")
            for d in range(DT):
                nc.sync.dma_start(wv_sb[:, d], wvT[d * 128 : (d + 1) * 128, :])
            wvb = consts.tile([128, DT, JW], bf16, tag="wvb")
            nc.vector.tensor_copy(wvb, wv_sb)
            for d in range(1, DT):
                for ch in range(NCH):
                    x = stage.tile([128, CHW], f32r, tag="xc", name="x")
                    nc.sync.dma_start(
                        x, kT[d * 128 : (d + 1) * 128, ch * CHW : (ch + 1) * CHW]
                    )
                    k_tiles[(d, ch)] = x
            q_tiles = dma_x_chunk(qT, 0)
            m8 = []
            for t in range(NKT):
                mt8 = m8pool.tile([128, NQ], u8, tag="m8", name="m8")
                nc.sync.dma_start(mt8, maskT[t * 128 : (t + 1) * 128, :])
                m8.append(mt8)
            v_tiles = {}
            for ch in range(NCH):
                dma_x_chunk(vT, ch, v_tiles)
            q_later = {}
            for ch in range(1, NCH):
                dma_x_chunk(qT, ch, q_later, pool=qpool)

            # ---- projections ----
            # qtT/ktT layout: partition 64h'+dh, free m -> head 2m+h'
            # fp8 path: kz[64h'+d, m, m', nk] = (m'==m) ? kt_{2m+h'} : 0 so a
            # DoubleRow matmul over both m' slots contracts only head 2m+h'
            # (the zero stationary slot kills the other head's q rows).
            qtT = projout.tile([128, 2, NQ], QK_DT, tag="qtT")
            if SCORES_FP8:
                ktT = projout.tile([128, 2, 2, NK], QK_DT, tag="ktT")
                for m in range(2):
                    nc.gpsimd.memset(ktT[:, m, 1 - m, :], 0)
            else:
                ktT = projout.tile([128, 2, NK], QK_DT, tag="ktT")

            def cast_x(tiles, d, ch, pool):
                xb = pool.tile([128, CHW], bf16, tag="xb", name="xb")
                # SWDGE cast DMA f32 -> bf16
                nc.gpsimd.dma_start(xb, tiles[(d, ch)])
                return xb

            def proj_k_full():
                s0 = sps.tile([128, 2, CHW], f32, tag="s", name="kp0")
                s1 = sps.tile([128, 2, CHW], f32, tag="s", name="kp1")
                pvt = pvps.tile([128, HPC, CHW], f32, tag="pv", name="kp2")
                regions = [
                    s0[:, 0], s0[:, 1], s1[:, 0], s1[:, 1],
                    pvt[:, 0], pvt[:, 1], pvt[:, 2], pvt[:, 3],
                ]
                for d in range(DT):
                    kb = {ch: cast_x(k_tiles, d, ch, kbfp) for ch in range(NCH)}
                    for m in range(2):
                        for ch in range(NCH):
                            nc.tensor.matmul(
                                regions[m * NCH + ch],
                                wkb[:, d, m * 128 : (m + 1) * 128],
                                kb[ch],
                                start=(d == 0),
                                stop=(d == DT - 1),
                            )
                for m in range(2):
                    for ch in range(NCH):
                        dst = (
                            ktT[:, m, m, ch * CHW : (ch + 1) * CHW]
                            if SCORES_FP8
                            else ktT[:, m, ch * CHW : (ch + 1) * CHW]
                        )
                        nc.vector.tensor_scalar_add(
                            dst,
                            regions[m * NCH + ch],
                            bk_sb[:, m : m + 1],
                        )

            def proj_q_chunk(tiles, ch):
                qp = sps.tile([128, 2, CHW], f32, tag="s", name="qp")
                for d in range(DT):
                    qb = cast_x(tiles, d, ch, kbfp)
                    for m in range(2):
                        nc.tensor.matmul(
                            qp[:, m],
                            wqb[:, d, m * 128 : (m + 1) * 128],
                            qb,
                            start=(d == 0),
                            stop=(d == DT - 1),
                        )
                for m in range(2):
                    nc.vector.tensor_scalar_add(
                        qtT[:, m, ch * CHW : (ch + 1) * CHW],
                        qp[:, m],
                        bq_sb[:, m : m + 1],
                    )

            def proj_v():
                vbf = {}
                for n in range(NKT):
                    ch, nn_ = divmod(n, NCH)
                    ps = sps.tile([128, 2, CHW], f32, tag="s", name="vps")
                    reg = ps[:, 0, 0:JW]
                    for d in range(DT):
                        if (d, ch) not in vbf:
                            vbf[(d, ch)] = cast_x(v_tiles, d, ch, vbfp)
                        nc.tensor.matmul(
                            reg,
                            vbf[(d, ch)][:, nn_ * 128 : (nn_ + 1) * 128],
                            wvb[:, d, :],
                            start=(d == 0),
                            stop=False,
                        )
                    # += ones^T @ bv  (fold the V bias into vt)
                    nc.tensor.matmul(
                        reg,
                        ones_sb[0:1, 0:128],
                        bvr_sb[0:1, :],
                        start=False,
                        stop=True,
                    )
                    dst = vt.rearrange("p n (h c) -> p n h c", c=65)[
                        :, n, :, 0:64
                    ]
                    nc.vector.tensor_copy(
                        dst, reg.rearrange("p (h c) -> p h c", c=64)
                    )

            # ---- attention ----
            def scores_group(hpair, t, ch, cs, p_t, mb_box):
                sp = sps.tile([128, 2, CHW], f32, tag="s", name="sp")
                for hh in range(2):
                    h = 2 * hpair + hh
                    hp, m = h % 2, h // 2
                    if SCORES_FP8:
                        nc.tensor.matmul(
                            sp[:, hh],
                            ktT[
                                64 * hp : 64 * (hp + 1),
                                m,
                                :,
                                t * 128 : (t + 1) * 128,
                            ],
                            qtT[64 * hp : 64 * (hp + 1), :, cs],
                            start=True,
                            stop=True,
                            perf_mode=mybir.MatmulPerfMode.DoubleRow,
                        )
                    else:
                        nc.tensor.matmul(
                            sp[:, hh],
                            ktT[64 * hp : 64 * (hp + 1), m, t * 128 : (t + 1) * 128],
                            qtT[64 * hp : 64 * (hp + 1), m, cs],
                            start=True,
                            stop=True,
                        )
                if hpair == 0:
                    mb = mbpool.tile([128, CHW], bf16, tag="mb", name="mb")
                    # SWDGE cast DMA u8 -> bf16
                    nc.gpsimd.dma_start(mb, m8[t][:, cs])
                    mb_box[t] = mb
                nc.scalar.activation(
                    out=p_t[:, 2 * hpair : 2 * hpair + 2, :],
                    in_=sp,
                    func=mybir.ActivationFunctionType.Exp,
                    scale=1.0 / 32.0,
                )

            def mask_mult(t, p_t, mb_box):
                mb = mb_box[t]
                nc.vector.tensor_mul(
                    p_t,
                    p_t,
                    mb.rearrange("p (a c) -> p a c", a=1).to_broadcast(
                        (128, HPC, CHW)
                    ),
                )

            def pv_t(t, p_t, pv_ps):
                for h in range(HPC):
                    nc.tensor.matmul(
                        pv_ps[0:65, h, :],
                        vt[:, t, 65 * h : 65 * h + 65],
                        p_t[:, h, :],
                        start=(t == 0),
                        stop=(t == NKT - 1),
                    )

            def dbg_dump(tens, src):
                for a in range(src.shape[1]):
                    t = dbgpool.tile(
                        [128, src.shape[2]], f32, tag="dbg", name="dbg"
                    )
                    nc.vector.tensor_copy(t, src[:, a])
                    w = src.shape[2]
                    nc.sync.dma_start(tens[:, a * w : (a + 1) * w], t)

            def chunk_tail(cs, pv_ps, ch=None):  # noqa: C901
                if KDBG and ch == 0:
                    dbg_dump(dbg_pv, pv_ps)
                # rowsums live in psum partition row 64 of each head's bank
                rs_sb = rcpool.tile([1, HPC * CHW], f32, tag="rs", name="rs")
                nc.vector.tensor_copy(
                    rs_sb, pv_ps[64:65, :, :].rearrange("p h c -> p (h c)")
                )
                rc = rcpool.tile([1, HPC * CHW], f32, tag="rc", name="rc")
                nc.vector.reciprocal_approx_fast(out=rc, in_=rs_sb)
                rcr = rcpool.tile([1, HPC * CHW], bf16, tag="rcr", name="rcr")
                nc.vector.tensor_copy(rcr, rc)
                if KDBG and ch == 0:
                    nc.sync.dma_start(dbg_rc[0:1, :], rc)
                for hp in range(2):
                    rb = rbpool.tile([128, 2 * CHW], f32, tag="rb", name="rb")
                    rbp = sps.tile([128, 2, CHW], f32, tag="s", name="rbp")
                    for hh in range(2):
                        h = 2 * hp + hh
                        nc.tensor.matmul(
                            rbp[0:64, hh, :],
                            ones_sb[0:1, 0:64],
                            rcr[0:1, h * CHW : (h + 1) * CHW],
                            start=True,
                            stop=True,
                        )
                    nc.vector.tensor_copy(
                        rb[0:64, :], rbp[0:64].rearrange("p h c -> p (h c)")
                    )
                    if KDBG and ch == 0 and hp == 0:
                        nc.sync.dma_start(dbg_rb[:], rb[0:64, :])
                    for hh in range(2):
                        h = 2 * hp + hh
                        osb = outsb.tile([128, CHW], f32, tag="o", name="osb")
                        nc.vector.tensor_mul(
                            osb[0:64, :],
                            pv_ps[0:64, h, :],
                            rb[0:64, hh * CHW : (hh + 1) * CHW],
                        )
                        nc.sync.dma_start(o[64 * h : 64 * (h + 1), cs], osb[0:64, :])

            proj_k_full()
            proj_q_chunk(q_tiles, 0)
            proj_v()
            if KDBG:
                for seg in range(8):
                    w = NKT * HPC * 65 // 8
                    t = dbgpool.tile([128, w], f32, tag="dbgv", name="dbgv")
                    nc.vector.tensor_copy(
                        t,
                        vt.rearrange("p n c -> p (n c)")[
                            :, seg * w : (seg + 1) * w
                        ],
                    )
                    nc.sync.dma_start(dbg_vt[:, seg * w : (seg + 1) * w], t)

            pending_tail = None
            for ch in range(NCH):
                cs = slice(ch * CHW, (ch + 1) * CHW)
                pv_ps = pvps.tile([128, HPC, CHW], f32, tag="pv", name="pv")
                mb_box = {}
                p_tiles = {}
                for t in range(NKT + LAG):
                    if t < NKT:
                        p_t = ppool.tile([128, HPC, CHW], bf16, tag="p", name="p")
                        p_tiles[t] = p_t
                        for hpair in range(2):
                            scores_group(hpair, t, ch, cs, p_t, mb_box)
                        mask_mult(t, p_t, mb_box)
                    if t == 1 and pending_tail is not None:
                        pending_tail()
                        pending_tail = None
                    if t >= LAG:
                        pt = p_tiles.pop(t - LAG)
                        if KDBG and ch == 0 and t - LAG == 0:
                            dbg_dump(dbg_p, pt)
                        pv_t(t - LAG, pt, pv_ps)
                if ch + 1 < NCH:
                    proj_q_chunk(q_later, ch + 1)

                def _tail(cs=cs, pv_ps=pv_ps, ch=ch):
                    chunk_tail(cs, pv_ps, ch)

                pending_tail = _tail
            pending_tail()

    nc.compile()
    return nc


_NC = None


def _get_nc():
    global _NC
    if _NC is None:
        _NC = _build()
    return _NC


def _w_perm():
    """Host-side permutation of the per-core 256 W rows.

    fp8: position m*128 + 32h + p  <- feature 64h + 32m + p
    bf16: position m*128 + 64h' + d <- feature 64*(2m+h') + d  (head = 2m+h')
    """
    perm = np.empty(JW, np.int64)
    for i in range(JW):
        m, r = divmod(i, 128)
        hp, d = divmod(r, 64)
        perm[i] = 64 * (2 * m + hp) + d
    return perm


def _head_of_pos():
    """head index for each of the 4 64-wide blocks of o's rows given the
    vt/scores head numbering h=0..3 (identity under both _w_perm layouts)."""
    return [0, 1, 2, 3]


def _shard(inputs):
    import ml_dtypes

    q, k, v = inputs["q"], inputs["k"], inputs["v"]
    mask = inputs["mask"]
    Wq, bq, Wk, bk, Wv, bv = (
        inputs[n] for n in ("Wq", "bq", "Wk", "bk", "Wv", "bv")
    )
    qT = [np.ascontiguousarray(np.asarray(q[b], np.float32).T) for b in range(B)]
    kT = [np.ascontiguousarray(np.asarray(k[b], np.float32).T) for b in range(B)]
    vT = [np.ascontiguousarray(np.asarray(v[b], np.float32).T) for b in range(B)]
    mT = [
        np.ascontiguousarray(np.asarray(mask[b]).T).view(np.uint8)
        for b in range(B)
    ]
    onesr = np.ones((1, 128), ml_dtypes.bfloat16)
    vtones = np.ones((128, NKT * HPC), ml_dtypes.bfloat16)
    perm = _w_perm()
    in_maps = []
    for c in range(N_CORES):
        b, jg = divmod(c, N_CORES // B)
        j0 = jg * JW
        Wqs = np.asarray(Wq, np.float32)[j0 : j0 + JW, :][perm]
        Wks = np.asarray(Wk, np.float32)[j0 : j0 + JW, :][perm]
        in_maps.append(
            {
                "qT": qT[b],
                "kT": kT[b],
                "vT": vT[b],
                "maskT": mT[b],
                "wqT": np.ascontiguousarray(Wqs.T),
                "wkT": np.ascontiguousarray(Wks.T),
                "wvT": np.ascontiguousarray(
                    np.asarray(Wv, np.float32)[j0 : j0 + JW, :].T
                ),
                "bq": np.asarray(bq, np.float32)[j0 : j0 + JW][perm].reshape(
                    2, 128
                ),
                "bk": np.asarray(bk, np.float32)[j0 : j0 + JW][perm].reshape(
                    2, 128
                ),
                "bvrow": np.asarray(bv, np.float32)[j0 : j0 + JW].reshape(1, JW),
                "onesr": onesr,
                "vtones": vtones,
            }
        )
    return in_maps


LAST_RESULT = None


def kernel(**inputs) -> np.ndarray:
    global LAST_RESULT
    nc = _get_nc()
    in_maps = _shard(inputs)
    trace = bool(int(os.environ.get("KTRACE", "0")))
    res = run_bass_kernel_spmd(
        nc,
        in_maps,
        core_ids=list(range(N_CORES)),
        trace=trace,
        trace_cores=[0] if trace else None,
    )
    LAST_RESULT = res
    hpos = _head_of_pos()
    out = np.empty((B, NQ, D), np.float32)
    for c in range(N_CORES):
        b, jg = divmod(c, N_CORES // B)
        j0 = jg * JW
        oc = res.results[c]["o"]  # [256, NQ]; rows 64h:64h+64 = head hpos[h]
        oh = oc.reshape(HPC, DH, NQ)
        for h in range(HPC):
            out[b, :, j0 + 64 * hpos[h] : j0 + 64 * hpos[h] + 64] = oh[h].T
    return out


if __name__ == "__main__":
    if os.environ.get("KBUILD_ONLY"):
        import tempfile

        from concourse.bass_utils import compile_bass_kernel

        nc = _build()
        with tempfile.TemporaryDirectory() as td:
            compile_bass_kernel(nc, td)
        print("BUILD+COMPILE OK")


# revision 62
# speedup vs baseline: 1.0778x; 1.0778x over previous
"""Trainium2 Bass kernel for nn_Attention_48498770706573.

Fused QKV-projection + masked softmax attention, sharded over 8 NeuronCores:
data-parallel over batch (B=2), tensor-parallel over heads (16 -> 4 per
core). Host does slicing/transposition/constant-upload only.

v2 design (vs baseline):
  - projections consume f32r DRAM tiles directly (no bf16 pre-casts on DVE)
  - scores in fp8e4m3 DoubleRow (K=64 packed as 32x2) -> 0.5 cycles/col
  - rowsum fused into PV as a 65th "ones" column of vt (no ones-matmuls)
  - V-bias folded into vt via a K=1 rank-1 matmul during the V projection
    (out = (pv + bv*rowsum)/rowsum == pv'/rowsum with v' = v + bv)
  - tail: reciprocal_approx_fast on the psum rowsum row, rank-1 broadcast
    matmul, one DVE multiply per head
  - mask stays a DVE broadcast multiply on bf16 p tiles (TensorTensor 2x)
"""

import os

import numpy as np

import concourse.bacc as bacc
import concourse.bass_utils as _bu
import concourse.mybir as mybir
import concourse.tile as tile
from concourse.bass_utils import run_bass_kernel_spmd

_ = _bu  # walrus --enable-ldw-opt=true rejects bass-emitted InstLdweights

B, NQ, NK, D, H = 2, 2048, 2048, 1024, 16
DH = D // H  # 64
N_CORES = 8
HPC = H // (N_CORES // B)  # heads per core = 4
JW = HPC * DH  # per-core projection width = 256
NKT = NK // 128  # 16 nk tiles
NCH = 4  # nq chunks
CHW = NQ // NCH  # 512
DT = 8  # contraction d-tiles
LAG = 3

SCORES_FP8 = True

f32 = mybir.dt.float32
f32r = mybir.dt.float32r
bf16 = mybir.dt.bfloat16
fp8 = mybir.dt.float8e4
u8 = mybir.dt.uint8

QK_DT = fp8 if SCORES_FP8 else bf16


def _build():
    nc = bacc.Bacc(
        "TRN2", target_bir_lowering=False, debug=False, num_devices=N_CORES
    )

    qT = nc.dram_tensor("qT", [D, NQ], f32r, kind="ExternalInput")
    kT = nc.dram_tensor("kT", [D, NK], f32r, kind="ExternalInput")
    vT = nc.dram_tensor("vT", [D, NK], f32r, kind="ExternalInput")
    maskT = nc.dram_tensor("maskT", [NK, NQ], u8, kind="ExternalInput")
    wqT = nc.dram_tensor("wqT", [D, JW], f32r, kind="ExternalInput")
    wkT = nc.dram_tensor("wkT", [D, JW], f32r, kind="ExternalInput")
    wvT = nc.dram_tensor("wvT", [D, JW], f32r, kind="ExternalInput")
    bqd = nc.dram_tensor("bq", [2, 128], f32, kind="ExternalInput")
    bkd = nc.dram_tensor("bk", [2, 128], f32, kind="ExternalInput")
    bvrd = nc.dram_tensor("bvrow", [1, JW], f32, kind="ExternalInput")
    onesd = nc.dram_tensor("onesr", [1, 128], bf16, kind="ExternalInput")
    vtonesd = nc.dram_tensor("vtones", [128, NKT * HPC], bf16, kind="ExternalInput")
    o = nc.dram_tensor("o", [JW, NQ], f32, kind="ExternalOutput")
    KDBG = bool(int(os.environ.get("KDBG", "0")))
    if KDBG:
        dbg_vt = nc.dram_tensor(
            "dbg_vt", [128, NKT * HPC * 65], f32, kind="ExternalOutput"
        )
        dbg_pv = nc.dram_tensor(
            "dbg_pv", [128, HPC * CHW], f32, kind="ExternalOutput"
        )
        dbg_p = nc.dram_tensor(
            "dbg_p", [128, HPC * CHW], f32, kind="ExternalOutput"
        )
        dbg_rc = nc.dram_tensor(
            "dbg_rc", [2, HPC * CHW], f32, kind="ExternalOutput"
        )
        dbg_rb = nc.dram_tensor(
            "dbg_rb", [64, 2 * CHW], f32, kind="ExternalOutput"
        )

    with tile.TileContext(nc) as tc:
        with (
            tc.tile_pool(name="consts", bufs=1) as consts,
            tc.tile_pool(name="stage", bufs=12) as stage,
            tc.tile_pool(name="qpool", bufs=11) as qpool,
            tc.tile_pool(name="m8pool", bufs=16) as m8pool,
            tc.tile_pool(name="mbpool", bufs=6) as mbpool,
            tc.tile_pool(name="projout", bufs=1) as projout,
            tc.tile_pool(name="ppool", bufs=4) as ppool,
            tc.tile_pool(name="rcpool", bufs=1) as rcpool,
            tc.tile_pool(name="rbpool", bufs=2) as rbpool,
            tc.tile_pool(name="outsb", bufs=4) as outsb,
            tc.tile_pool(name="dbgpool", bufs=2) as dbgpool,
            tc.tile_pool(name="vbfp", bufs=10) as vbfp,
            tc.tile_pool(name="kbfp", bufs=10) as kbfp,
            tc.tile_pool(name="wtmp", bufs=1) as wtmp,
            tc.tile_pool(name="sps", bufs=2, space="PSUM") as sps,
            tc.tile_pool(name="pvps", bufs=1, space="PSUM") as pvps,
        ):
            # ---- constants ----
            wk_sb = wtmp.tile([128, DT, JW], f32r, tag="wt", name="wk_sb")
            for d in range(DT):
                nc.sync.dma_start(wk_sb[:, d], wkT[d * 128 : (d + 1) * 128, :])
            bq_sb = consts.tile([128, 2], f32, tag="bq")
            bk_sb = consts.tile([128, 2], f32, tag="bk")
            for m in range(2):
                nc.sync.dma_start(
                    bq_sb[:, m : m + 1],
                    bqd[m : m + 1, :].rearrange("a b -> b a"),
                )
                nc.sync.dma_start(
                    bk_sb[:, m : m + 1],
                    bkd[m : m + 1, :].rearrange("a b -> b a"),
                )
            bvrf = wtmp.tile([1, JW], f32, tag="bvrf", name="bvrf")
            nc.sync.dma_start(bvrf, bvrd[:])
            bvr_sb = consts.tile([1, JW], bf16, tag="bvr")
            nc.vector.tensor_copy(bvr_sb, bvrf)
            ones_sb = consts.tile([1, 128], bf16, tag="ones")
            nc.sync.dma_start(ones_sb, onesd[:])
            # bf16 copies of wq/wk for PE stationaries (ldw-opt compatible)
            wkb = consts.tile([128, DT, JW], bf16, tag="wkb")
            nc.vector.tensor_copy(wkb, wk_sb)

            # vt: [128, NKT, 4*65]; col 65h+64 is the ones column (rowsum)
            vt = projout.tile([128, NKT, HPC * 65], bf16, tag="vt")
            vt_ones_view = vt.rearrange("p n (h c) -> p n h c", c=65)[
                :, :, :, 64:65
            ].rearrange("p n h c -> p (n h c)")
            nc.sync.dma_start(vt_ones_view, vtonesd[:])

            # ---- decoupled input DMAs (priority order) ----
            # Load-phase DMAs alternate between the two HWDGE queues (SP and
            # Activation) to halve the serialized transfer stream; the ACT
            # queue is idle until the first exp anyway.
            _dma_rr = [0]

            def dma_in(dst, src):
                eng = nc.sync if _dma_rr[0] % 2 == 0 else nc.scalar
                _dma_rr[0] += 1
                eng.dma_start(dst, src)

            def dma_x_chunk(src, ch, tiles=None, pool=None):
                pool = pool or stage
                tiles = {} if tiles is None else tiles
                for d in range(DT):
                    x = pool.tile([128, CHW], f32r, tag="xc", name="x")
                    dma_in(
                        x,
                        src[d * 128 : (d + 1) * 128, ch * CHW : (ch + 1) * CHW],
                    )
                    tiles[(d, ch)] = x
                return tiles

            k_tiles = {}
            for ch in range(NCH):
                x = stage.tile([128, CHW], f32r, tag="xc", name="x")
                dma_in(x, kT[0:128, ch * CHW : (ch + 1) * CHW])
                k_tiles[(0, ch)] = x
            wq_sb = wtmp.tile([128, DT, JW], f32r, tag="wt", name="wq_sb")
            for d in range(DT):
                dma_in(wq_sb[:, d], wqT[d * 128 : (d + 1) * 128, :])
            wqb = consts.tile([128, DT, JW], bf16, tag="wqb")
            nc.vector.tensor_copy(wqb, wq_sb)
            wv_sb = wtmp.tile([128, DT, JW], f32r, tag="wt", name="wv_sb")
            for d in range(DT):
                dma_in(wv_sb[:, d], wvT[d * 128 : (d + 1) * 128, :])
            wvb = consts.tile([128, DT, JW], bf16, tag="wvb")
            nc.vector.tensor_copy(wvb, wv_sb)
            for d in range(1, DT):
                for ch in range(NCH):
                    x = stage.tile([128, CHW], f32r, tag="xc", name="x")
                    dma_in(
                        x, kT[d * 128 : (d + 1) * 128, ch * CHW : (ch + 1) * CHW]
                    )
                    k_tiles[(d, ch)] = x
            q_tiles = dma_x_chunk(qT, 0)
            m8 = []
            for t in range(NKT):
                mt8 = m8pool.tile([128, NQ], u8, tag="m8", name="m8")
                m8.append(mt8)
            for t in range(2):
                dma_in(m8[t], maskT[t * 128 : (t + 1) * 128, :])
            v_tiles = {}
            for ch in range(NCH):
                dma_x_chunk(vT, ch, v_tiles)
                for t in range(2 + 3 * ch, 2 + 3 * (ch + 1)):
                    dma_in(m8[t], maskT[t * 128 : (t + 1) * 128, :])
            for t in range(14, NKT):
                dma_in(m8[t], maskT[t * 128 : (t + 1) * 128, :])
            q_later = {}
            for ch in range(1, NCH):
                dma_x_chunk(qT, ch, q_later, pool=qpool)

            # ---- projections ----
            # qtT/ktT layout: partition 64h'+dh, free m -> head 2m+h'
            # fp8 path: kz[64h'+d, m, m', nk] = (m'==m) ? kt_{2m+h'} : 0 so a
            # DoubleRow matmul over both m' slots contracts only head 2m+h'
            # (the zero stationary slot kills the other head's q rows).
            qtT = projout.tile([128, 2, NQ], QK_DT, tag="qtT")
            if SCORES_FP8:
                ktT = projout.tile([128, 2, 2, NK], QK_DT, tag="ktT")
                for m in range(2):
                    nc.gpsimd.memset(ktT[:, m, 1 - m, :], 0)
            else:
                ktT = projout.tile([128, 2, NK], QK_DT, tag="ktT")

            def cast_x(tiles, d, ch, pool, swdge=False):
                xb = pool.tile([128, CHW], bf16, tag="xb", name="xb")
                if swdge:
                    # SWDGE cast DMA f32 -> bf16 (during attention: DVE busy)
                    nc.gpsimd.dma_start(xb, tiles[(d, ch)])
                else:
                    # load phase: DVE is idle
                    nc.vector.tensor_copy(xb, tiles[(d, ch)])
                return xb

            def proj_k_full():
                s0 = sps.tile([128, 2, CHW], f32, tag="s", name="kp0")
                s1 = sps.tile([128, 2, CHW], f32, tag="s", name="kp1")
                pvt = pvps.tile([128, HPC, CHW], f32, tag="pv", name="kp2")
                regions = [
                    s0[:, 0], s0[:, 1], s1[:, 0], s1[:, 1],
                    pvt[:, 0], pvt[:, 1], pvt[:, 2], pvt[:, 3],
                ]
                for d in range(DT):
                    kb = {ch: cast_x(k_tiles, d, ch, kbfp) for ch in range(NCH)}
                    for m in range(2):
                        for ch in range(NCH):
                            nc.tensor.matmul(
                                regions[m * NCH + ch],
                                wkb[:, d, m * 128 : (m + 1) * 128],
                                kb[ch],
                                start=(d == 0),
                                stop=(d == DT - 1),
                            )
                for m in range(2):
                    for ch in range(NCH):
                        dst = (
                            ktT[:, m, m, ch * CHW : (ch + 1) * CHW]
                            if SCORES_FP8
                            else ktT[:, m, ch * CHW : (ch + 1) * CHW]
                        )
                        nc.vector.tensor_scalar_add(
                            dst,
                            regions[m * NCH + ch],
                            bk_sb[:, m : m + 1],
                        )

            def proj_q_chunk(tiles, ch):
                qp = sps.tile([128, 2, CHW], f32, tag="s", name="qp")
                for d in range(DT):
                    qb = cast_x(tiles, d, ch, kbfp, swdge=(ch > 0))
                    for m in range(2):
                        nc.tensor.matmul(
                            qp[:, m],
                            wqb[:, d, m * 128 : (m + 1) * 128],
                            qb,
                            start=(d == 0),
                            stop=(d == DT - 1),
                        )
                for m in range(2):
                    nc.vector.tensor_scalar_add(
                        qtT[:, m, ch * CHW : (ch + 1) * CHW],
                        qp[:, m],
                        bq_sb[:, m : m + 1],
                    )

            _vbf = {}

            def proj_v_tile(n):
                ch, nn_ = divmod(n, NCH)
                ps = sps.tile([128, 2, CHW], f32, tag="s", name="vps")
                reg = ps[:, 0, 0:JW]
                for d in range(DT):
                    if (d, ch) not in _vbf:
                        _vbf[(d, ch)] = cast_x(v_tiles, d, ch, vbfp)
                    nc.tensor.matmul(
                        reg,
                        _vbf[(d, ch)][:, nn_ * 128 : (nn_ + 1) * 128],
                        wvb[:, d, :],
                        start=(d == 0),
                        stop=False,
                    )
                # += ones^T @ bv  (fold the V bias into vt)
                nc.tensor.matmul(
                    reg,
                    ones_sb[0:1, 0:128],
                    bvr_sb[0:1, :],
                    start=False,
                    stop=True,
                )
                dst = vt.rearrange("p n (h c) -> p n h c", c=65)[:, n, :, 0:64]
                nc.vector.tensor_copy(
                    dst, reg.rearrange("p (h c) -> p h c", c=64)
                )

            # ---- attention ----
            def scores_group(hpair, t, ch, cs, p_t, mb_box):
                sp = sps.tile([128, 2, CHW], f32, tag="s", name="sp")
                for hh in range(2):
                    h = 2 * hpair + hh
                    hp, m = h % 2, h // 2
                    if SCORES_FP8:
                        nc.tensor.matmul(
                            sp[:, hh],
                            ktT[
                                64 * hp : 64 * (hp + 1),
                                m,
                                :,
                                t * 128 : (t + 1) * 128,
                            ],
                            qtT[64 * hp : 64 * (hp + 1), :, cs],
                            start=True,
                            stop=True,
                            perf_mode=mybir.MatmulPerfMode.DoubleRow,
                        )
                    else:
                        nc.tensor.matmul(
                            sp[:, hh],
                            ktT[64 * hp : 64 * (hp + 1), m, t * 128 : (t + 1) * 128],
                            qtT[64 * hp : 64 * (hp + 1), m, cs],
                            start=True,
                            stop=True,
                        )
                if hpair == 0:
                    mb = mbpool.tile([128, CHW], bf16, tag="mb", name="mb")
                    # SWDGE cast DMA u8 -> bf16
                    nc.gpsimd.dma_start(mb, m8[t][:, cs])
                    mb_box[t] = mb
                nc.scalar.activation(
                    out=p_t[:, 2 * hpair : 2 * hpair + 2, :],
                    in_=sp,
                    func=mybir.ActivationFunctionType.Exp,
                    scale=1.0 / 32.0,
                )

            def mask_mult(t, p_t, mb_box):
                mb = mb_box[t]
                nc.vector.tensor_mul(
                    p_t,
                    p_t,
                    mb.rearrange("p (a c) -> p a c", a=1).to_broadcast(
                        (128, HPC, CHW)
                    ),
                )

            def pv_t(t, p_t, pv_ps):
                for h in range(HPC):
                    nc.tensor.matmul(
                        pv_ps[0:65, h, :],
                        vt[:, t, 65 * h : 65 * h + 65],
                        p_t[:, h, :],
                        start=(t == 0),
                        stop=(t == NKT - 1),
                    )

            def dbg_dump(tens, src):
                for a in range(src.shape[1]):
                    t = dbgpool.tile(
                        [128, src.shape[2]], f32, tag="dbg", name="dbg"
                    )
                    nc.vector.tensor_copy(t, src[:, a])
                    w = src.shape[2]
                    nc.sync.dma_start(tens[:, a * w : (a + 1) * w], t)

            def chunk_tail(cs, pv_ps, ch=None):  # noqa: C901
                if KDBG and ch == 0:
                    dbg_dump(dbg_pv, pv_ps)
                # rowsums live in psum partition row 64 of each head's bank
                rs_sb = rcpool.tile([1, HPC * CHW], f32, tag="rs", name="rs")
                nc.vector.tensor_copy(
                    rs_sb, pv_ps[64:65, :, :].rearrange("p h c -> p (h c)")
                )
                rc = rcpool.tile([1, HPC * CHW], f32, tag="rc", name="rc")
                nc.vector.reciprocal_approx_fast(out=rc, in_=rs_sb)
                rcr = rcpool.tile([1, HPC * CHW], bf16, tag="rcr", name="rcr")
                nc.vector.tensor_copy(rcr, rc)
                if KDBG and ch == 0:
                    nc.sync.dma_start(dbg_rc[0:1, :], rc)
                for hp in range(2):
                    rb = rbpool.tile([128, 2 * CHW], f32, tag="rb", name="rb")
                    rbp = sps.tile([128, 2, CHW], f32, tag="s", name="rbp")
                    for hh in range(2):
                        h = 2 * hp + hh
                        nc.tensor.matmul(
                            rbp[0:64, hh, :],
                            ones_sb[0:1, 0:64],
                            rcr[0:1, h * CHW : (h + 1) * CHW],
                            start=True,
                            stop=True,
                        )
                    nc.vector.tensor_copy(
                        rb[0:64, :], rbp[0:64].rearrange("p h c -> p (h c)")
                    )
                    if KDBG and ch == 0 and hp == 0:
                        nc.sync.dma_start(dbg_rb[:], rb[0:64, :])
                    for hh in range(2):
                        h = 2 * hp + hh
                        osb = outsb.tile([128, CHW], f32, tag="o", name="osb")
                        nc.vector.tensor_mul(
                            osb[0:64, :],
                            pv_ps[0:64, h, :],
                            rb[0:64, hh * CHW : (hh + 1) * CHW],
                        )
                        nc.sync.dma_start(o[64 * h : 64 * (h + 1), cs], osb[0:64, :])

            proj_k_full()
            proj_q_chunk(q_tiles, 0)

            pending_tail = None
            for ch in range(NCH):
                cs = slice(ch * CHW, (ch + 1) * CHW)
                pv_ps = pvps.tile([128, HPC, CHW], f32, tag="pv", name="pv")
                mb_box = {}
                p_tiles = {}
                for t in range(NKT + LAG):
                    if t < NKT:
                        p_t = ppool.tile([128, HPC, CHW], bf16, tag="p", name="p")
                        p_tiles[t] = p_t
                        for hpair in range(2):
                            scores_group(hpair, t, ch, cs, p_t, mb_box)
                        mask_mult(t, p_t, mb_box)
                        if ch == 0:
                            # v-projection rides along with chunk 0's scores;
                            # vt tile t is ready LAG steps before pv needs it
                            proj_v_tile(t)
                    if t == 1 and pending_tail is not None:
                        pending_tail()
                        pending_tail = None
                    if t >= LAG:
                        pt = p_tiles.pop(t - LAG)
                        if KDBG and ch == 0 and t - LAG == 0:
                            dbg_dump(dbg_p, pt)
                        pv_t(t - LAG, pt, pv_ps)
                if ch + 1 < NCH:
                    proj_q_chunk(q_later, ch + 1)

                def _tail(cs=cs, pv_ps=pv_ps, ch=ch):
                    chunk_tail(cs, pv_ps, ch)

                pending_tail = _tail
            pending_tail()
            if KDBG:
                for seg in range(8):
                    w = NKT * HPC * 65 // 8
                    t = dbgpool.tile([128, w], f32, tag="dbgv", name="dbgv")
                    nc.vector.tensor_copy(
                        t,
                        vt.rearrange("p n c -> p (n c)")[
                            :, seg * w : (seg + 1) * w
                        ],
                    )
                    nc.sync.dma_start(dbg_vt[:, seg * w : (seg + 1) * w], t)

    nc.compile()
    return nc


_NC = None


def _get_nc():
    global _NC
    if _NC is None:
        _NC = _build()
    return _NC


def _w_perm():
    """Host-side permutation of the per-core 256 W rows.

    fp8: position m*128 + 32h + p  <- feature 64h + 32m + p
    bf16: position m*128 + 64h' + d <- feature 64*(2m+h') + d  (head = 2m+h')
    """
    perm = np.empty(JW, np.int64)
    for i in range(JW):
        m, r = divmod(i, 128)
        hp, d = divmod(r, 64)
        perm[i] = 64 * (2 * m + hp) + d
    return perm


def _head_of_pos():
    """head index for each of the 4 64-wide blocks of o's rows given the
    vt/scores head numbering h=0..3 (identity under both _w_perm layouts)."""
    return [0, 1, 2, 3]


def _shard(inputs):
    import ml_dtypes

    q, k, v = inputs["q"], inputs["k"], inputs["v"]
    mask = inputs["mask"]
    Wq, bq, Wk, bk, Wv, bv = (
        inputs[n] for n in ("Wq", "bq", "Wk", "bk", "Wv", "bv")
    )
    qT = [np.ascontiguousarray(np.asarray(q[b], np.float32).T) for b in range(B)]
    kT = [np.ascontiguousarray(np.asarray(k[b], np.float32).T) for b in range(B)]
    vT = [np.ascontiguousarray(np.asarray(v[b], np.float32).T) for b in range(B)]
    mT = [
        np.ascontiguousarray(np.asarray(mask[b]).T).view(np.uint8)
        for b in range(B)
    ]
    onesr = np.ones((1, 128), ml_dtypes.bfloat16)
    vtones = np.ones((128, NKT * HPC), ml_dtypes.bfloat16)
    perm = _w_perm()
    in_maps = []
    for c in range(N_CORES):
        b, jg = divmod(c, N_CORES // B)
        j0 = jg * JW
        Wqs = np.asarray(Wq, np.float32)[j0 : j0 + JW, :][perm]
        Wks = np.asarray(Wk, np.float32)[j0 : j0 + JW, :][perm]
        in_maps.append(
            {
                "qT": qT[b],
                "kT": kT[b],
                "vT": vT[b],
                "maskT": mT[b],
                "wqT": np.ascontiguousarray(Wqs.T),
                "wkT": np.ascontiguousarray(Wks.T),
                "wvT": np.ascontiguousarray(
                    np.asarray(Wv, np.float32)[j0 : j0 + JW, :].T
                ),
                "bq": np.asarray(bq, np.float32)[j0 : j0 + JW][perm].reshape(
                    2, 128
                ),
                "bk": np.asarray(bk, np.float32)[j0 : j0 + JW][perm].reshape(
                    2, 128
                ),
                "bvrow": np.asarray(bv, np.float32)[j0 : j0 + JW].reshape(1, JW),
                "onesr": onesr,
                "vtones": vtones,
            }
        )
    return in_maps


LAST_RESULT = None


def kernel(**inputs) -> np.ndarray:
    global LAST_RESULT
    nc = _get_nc()
    in_maps = _shard(inputs)
    trace = bool(int(os.environ.get("KTRACE", "0")))
    res = run_bass_kernel_spmd(
        nc,
        in_maps,
        core_ids=list(range(N_CORES)),
        trace=trace,
        trace_cores=[0] if trace else None,
    )
    LAST_RESULT = res
    hpos = _head_of_pos()
    out = np.empty((B, NQ, D), np.float32)
    for c in range(N_CORES):
        b, jg = divmod(c, N_CORES // B)
        j0 = jg * JW
        oc = res.results[c]["o"]  # [256, NQ]; rows 64h:64h+64 = head hpos[h]
        oh = oc.reshape(HPC, DH, NQ)
        for h in range(HPC):
            out[b, :, j0 + 64 * hpos[h] : j0 + 64 * hpos[h] + 64] = oh[h].T
    return out


if __name__ == "__main__":
    if os.environ.get("KBUILD_ONLY"):
        import tempfile

        from concourse.bass_utils import compile_bass_kernel

        nc = _build()
        with tempfile.TemporaryDirectory() as td:
            compile_bass_kernel(nc, td)
        print("BUILD+COMPILE OK")
